# revision 23
# baseline (speedup 1.0000x reference)
"""MoE hard-routing kernel for Trainium2 (8 NeuronCores, Bass/Tile).

Problem: out[t] = x[t] @ W[p[t]].T + b[p[t]]
  x [8, 4096, 512] f32, partitions [8, 4096] int32 (values 0..7),
  W [8, 512, 512] f32, b [8, 512] f32.

Strategy: expert-parallel sharding. n_experts == n_cores == 8, so core e
owns expert e. The host routes each token to its expert's core (that IS the
shard assignment — a partition of the token set), pre-transposed so d_in
lies on SBUF partitions, and pre-cast to fp16 (tolerance is 2e-2; fp16
operands keep the GEMM rel-err ~1e-3 while halving DMA traffic and keeping
the PE at its full 1 row/cycle rate). Each core runs one dense GEMM
  out_e[d_out, tok] = W[e] @ xT_e  (+ b[e])
accumulated over 4 K-chunks of 128 in fp32 PSUM, with the bias added and
the fp32->fp16 downcast fused into the PSUM eviction.

Timeline model (from NTFF traces): the NEFF runtime preamble (engine
bring-up, ucode fetch, $S[2] barriers) owns t=0..~5.9us and is untouchable;
the steady-state matmul cadence is already at the 216ns/512-col roofline;
so the wins are at the edges:
  - The bass-init all-engine entry barrier is stripped post-build: each
    engine then flows straight from the runtime preamble into its stream,
    so the first DMA triggers issue at ~6.1us and nobody waits for the
    slowest engine's block-0 work.
  - A few warm-up matmuls on uninitialized SBUF (no sem waits, results
    discarded) keep the PE busy from ~6.0us so the HAM clock-gate window
    (3.4us of sustained activity) elapses during the DMA bootstrap and the
    real matmuls run at 2.4 GHz as early as possible.
  - W is loaded in dependency order (k0/m01 first, 64KB) so the first real
    matmul's LDWEIGHTS unblocks ~1us earlier than a monolithic W load.
  - x is fully prefetched: blocks 0-1 per-K-chunk (the k'th matmul group
    waits only on its own 128KB chunk), later blocks one 512KB trigger
    each, all buffered in SBUF (no pool-recycle waits on the SP queue).
  - PSUM evictions alternate DVE (even m) / ACT (odd m); the last block is
    forced <=256 columns and its evictions/stores are split across both
    engines and both HWDGE rings so the tail drains fast.
"""

import sys

for _p in ("/opt/trn_rl_repo", "/root/.axon_site/_ro/trn_rl_repo"):
    if _p not in sys.path:
        sys.path.append(_p)

import numpy as np

import concourse.bass as bass
import concourse.mybir as mybir
import concourse.tile as tile
from concourse.bass import ts
from concourse.bass_utils import run_bass_kernel_spmd
import bass_rust as _br

D_IN = 512
D_OUT = 512
N_EXPERTS = 8
N_CORES = 8
P = 128
NBLK = 512  # token columns per matmul (one PSUM bank of fp32)
KC = D_IN // P  # 4 contraction chunks
MC = D_OUT // P  # 4 output-row chunks

MATH_MODE = "f16"

N_WARMUP = 6  # 512-col warm-up matmuls bridging engine-start -> first x chunk.
# Deliberately overshoots the expected x/W arrival (~9.3us vs warm-ups ending
# ~10.1us): a idle splice gap between warm-ups and the first real matmul
# breaks the HAM activity window and extends the 1.2 GHz cold throttle by
# 3.4-7us, far worse than ~0.5us of redundant warm-up.


def _np_dt(math_mode: str):
    if math_mode == "f16":
        return np.float16
    if math_mode == "bf16":
        import ml_dtypes

        return ml_dtypes.bfloat16
    if math_mode in ("f32r", "f32"):
        return np.float32
    raise ValueError(math_mode)


def _mm_dt(math_mode: str):
    return {
        "f16": mybir.dt.float16,
        "bf16": mybir.dt.bfloat16,
        "f32r": mybir.dt.float32r,
        "f32": mybir.dt.float32,
    }[math_mode]


def _split_multiwait(nc: bass.Bass) -> None:
    """Hoist extra sem waits onto injected same-engine nops.

    The walrus build in this container rejects more than one sync-wait
    command on a single instruction.  Engine queues are in-order, so a
    nop carrying one wait immediately before the instruction is
    semantically identical to the wait being attached directly.
    """
    cnt = 0
    for bb in nc.main_func.blocks:
        new = []
        changed = False
        for ins in bb.instructions:
            si = ins.sync_info
            if si is not None and len(si.on_wait) > 1:
                waits = list(si.on_wait)
                for w in waits[:-1]:
                    nop = mybir.InstNoOp(name=f"wsplit-{cnt}", ins=[], outs=[])
                    cnt += 1
                    nop.engine = ins.engine
                    nop.sync_info = _br.SyncInfo(on_wait=[w], on_update=[])
                    new.append(nop)
                ins.sync_info = _br.SyncInfo(
                    on_wait=[waits[-1]], on_update=list(si.on_update)
                )
                changed = True
            new.append(ins)
        if changed:
            bb.instructions = new


def _strip_entry_barrier(nc: bass.Bass) -> None:
    """Remove the bass-init all-engine barrier from block 0.

    Every cross-engine dependency in the tile body is expressed through
    semaphores, so the barrier only serializes the engines' entry into the
    body behind the slowest engine's block-0 work.  Dropping it lets the
    first DMA triggers and warm-up matmuls issue as soon as each engine
    clears the (fixed) runtime preamble.  The exit-side barriers and the
    semaphore range-clear are left untouched (they guard re-execution).
    """
    bb = nc.main_func.blocks[0]

    def _is_entry_barrier(ins) -> bool:
        if type(ins).__name__ not in ("InstDrain", "InstEventSemaphore"):
            return False
        si = ins.sync_info
        refs = []
        if si is not None:
            refs = list(si.on_wait) + list(si.on_update)
        if any("barrier_" in (r.ant_name or "") for r in refs):
            return True
        return "barrier_" in (getattr(ins, "name", "") or "")

    bb.instructions = [i for i in bb.instructions if not _is_entry_barrier(i)]


def _col_blocks(C: int) -> list:
    """Column superblocks: a 256-col first block (its per-K x chunks are
    64KB, so the first matmul's data lands ~0.5us sooner and the cold
    bootstrap pipelines finer), 512-col steady blocks, and a final block
    forced <=256 columns so the tail's evictions + stores drain quickly
    after the last matmul."""
    blocks = []
    off = 0
    while C - off > 640:
        blocks.append((off, NBLK))
        off += NBLK
    rem = C - off
    if rem > 256:
        blocks.append((off, rem - 128))
        blocks.append((off + rem - 128, 128))
    else:
        blocks.append((off, rem))
    return blocks


def _build_nc(C: int, math_mode: str) -> bass.Bass:
    """One core's program: out[512, C] = wT.T-contract(xT) + bias."""
    f32 = mybir.dt.float32
    nc = bass.Bass("TRN2", target_bir_lowering=False, debug=False, num_devices=N_CORES)

    mm_dt = _mm_dt(math_mode)
    out_dt = f32 if math_mode in ("f32r", "f32") else mm_dt

    xT = nc.declare_dram_parameter("xT", [D_IN, C], mm_dt, isOutput=False)
    wT = nc.declare_dram_parameter("wT", [D_IN, D_OUT], mm_dt, isOutput=False)
    bias = nc.declare_dram_parameter("bias", [D_OUT], f32, isOutput=False)
    out = nc.declare_dram_parameter("out", [D_OUT, C], out_dt, isOutput=True)

    # [p, k, c] views of the DRAM operands (k-chunk on the free dims).
    xT_v = xT.rearrange("(k p) c -> p k c", p=P)
    out_v = out.rearrange("(m p) c -> p m c", p=P)
    wT_v = wT.rearrange("(k p) d -> p k d", p=P)

    col_blocks = _col_blocks(C)
    n_blocks = len(col_blocks)

    # Warm-up source: a raw (non-tile-tracked) SBUF region read as garbage.
    # Contents are irrelevant — the PSUM scratch the warm-ups write is
    # overwritten by start=True matmuls later — and skipping the memset
    # means the warm-up matmuls carry NO waits at all: the PE is busy from
    # the instant it clears the runtime preamble.  The HAM clock-gate needs
    # ~3.4us of *uninterrupted* activity to lift the 1.2 GHz cold throttle,
    # so the warm-ups must bridge seamlessly into the first real matmul.
    warm_raw = nc.alloc_sbuf_tensor("warm_raw", [P, NBLK], mm_dt)
    warm_ap = warm_raw.ap()

    with tile.TileContext(nc) as tc:
        with (
            tc.tile_pool(name="wpool", bufs=1) as wpool,
            tc.tile_pool(name="xkpool", bufs=8) as xkpool,
            tc.tile_pool(name="xbpool", bufs=max(1, n_blocks - 2)) as xbpool,
            tc.tile_pool(name="opool", bufs=n_blocks) as opool,
            tc.tile_pool(name="pspool", bufs=8, space="PSUM") as pspool,
        ):
            for wi in range(N_WARMUP):
                ps = pspool.tile([P, NBLK], f32, name=f"ps_w{wi}", tag="ps")
                nc.tensor.matmul(
                    ps[:], warm_ap[:, :P], warm_ap[:, :], start=True, stop=True
                )

            # Ring schedule (both HWDGE rings drain round-robin per packet,
            # so each ring's own FIFO order IS the priority order):
            #   SP : W k0..k3 | bias | x b2 | x b3 | ... | late-half stores
            #   ACT: x b0 k0..k3 | x b1 k0..k3 | table preload | early stores
            # The two transfers gating the first real matmul (W k0, x b0 k0)
            # are each FIRST on their ring; the remaining W chunks stay ~1.5
            # k-groups ahead of the cold matmul stream, and the bulk x
            # prefetch queues behind W so it can never starve it.
            w_t = wpool.tile([P, KC, D_OUT], mm_dt)
            for k in range(KC):
                nc.sync.dma_start(w_t[:, k, :], wT_v[:, k, :])
            b_t = wpool.tile([P, MC], f32)
            nc.sync.dma_start(b_t[:], bias.rearrange("(m p) -> p m", p=P))
            # Preload the ACT activation table (Identity) so the first real
            # eviction doesn't pay the lazy 1.3us ACT_TABLE_LOAD.  Reads the
            # warm-up garbage; writes a scratch tile nobody reads.
            warm_o = wpool.tile([P, 1], f32)
            nc.scalar.activation(
                warm_o[:],
                warm_ap[:, :1],
                mybir.ActivationFunctionType.Identity,
                bias=0.0,
            )

            for n, (coff, csz) in enumerate(col_blocks):
                is_last = n == n_blocks - 1
                is_tail = n >= n_blocks - 2
                # x prefetch free-runs (every block has its own SBUF tiles,
                # no pool-recycle pacing): blocks 0-1 load per-K-chunk so
                # the bootstrap's k'th matmul group waits only on its own
                # 128KB transfer; later blocks are one 512KB trigger each
                # (triggers cost ~650ns of SP queue occupancy; fewer
                # triggers lets the whole prefetch finish by ~20us, freeing
                # the SP ring for the second half's stores).
                if n < 2:
                    x_k = []
                    for k in range(KC):
                        xt = xkpool.tile(
                            [P, NBLK], mm_dt, name=f"x_{n}_{k}", tag="xchunk"
                        )
                        nc.scalar.dma_start(
                            xt[:, :csz], xT_v[:, k, coff : coff + csz]
                        )
                        x_k.append(xt)
                    xs = lambda k, _x=x_k: _x[k][:, :csz]
                else:
                    xt = xbpool.tile(
                        [P, KC, NBLK], mm_dt, name=f"x_{n}", tag="xblock"
                    )
                    nc.sync.dma_start(xt[:, :, :csz], xT_v[:, :, coff : coff + csz])
                    xs = lambda k, _x=xt: _x[:, k, :csz]
                o_t = opool.tile([P, MC, csz], out_dt, name=f"o_{n}", tag="o")
                ps_m = [
                    pspool.tile([P, NBLK], f32, name=f"ps_{n}_{m}", tag="ps")
                    for m in range(MC)
                ]
                for k in range(KC):
                    for m in range(MC):
                        nc.tensor.matmul(
                            ps_m[m][:, :csz],
                            w_t[:, k, ts(m, P)],
                            xs(k),
                            start=(k == 0),
                            stop=(k == KC - 1),
                        )
                for m in range(MC):
                    # Steady state alternates DVE (even m) / ACT (odd m).
                    # The final two blocks pair DVE=m0,m1 / ACT=m2,m3 instead
                    # so each half-store below waits on only one engine's
                    # eviction chain.
                    on_act = (m >= 2) if is_tail else (m % 2 == 1)
                    if on_act:
                        nc.scalar.activation(
                            o_t[:, m, :csz],
                            ps_m[m][:, :csz],
                            mybir.ActivationFunctionType.Identity,
                            bias=b_t[:, m : m + 1],
                        )
                    else:
                        nc.vector.tensor_scalar_add(
                            o_t[:, m, :csz],
                            ps_m[m][:, :csz],
                            b_t[:, m : m + 1],
                        )
                # One store DMA per superblock.  Early blocks store on the
                # ACT ring (the SP ring is still streaming x); blocks whose
                # evictions happen after the x prefetch has drained store on
                # the now-idle SP ring, so neither ring builds the backlog
                # that used to push the final store receipt ~2us past the
                # last matmul.  The last two blocks split across both rings.
                if is_tail:
                    nc.sync.dma_start(
                        out_v[:, :2, coff : coff + csz], o_t[:, :2, :csz]
                    )
                    nc.scalar.dma_start(
                        out_v[:, 2:, coff : coff + csz], o_t[:, 2:, :csz]
                    )
                elif n < 4:
                    nc.scalar.dma_start(
                        out_v[:, :, coff : coff + csz], o_t[:, :, :csz]
                    )
                else:
                    nc.sync.dma_start(
                        out_v[:, :, coff : coff + csz], o_t[:, :, :csz]
                    )
    _split_multiwait(nc)
    _strip_entry_barrier(nc)
    return nc


_NC_CACHE: dict = {}


def _get_nc(C: int, math_mode: str) -> bass.Bass:
    key = (C, math_mode)
    if key not in _NC_CACHE:
        _NC_CACHE[key] = _build_nc(C, math_mode)
    return _NC_CACHE[key]


def kernel(x: np.ndarray, partitions: np.ndarray, W: np.ndarray, b: np.ndarray,
           _math_mode: str | None = None, _trace: bool = False):
    math_mode = _math_mode or MATH_MODE
    np_dt = _np_dt(math_mode)
    B, S, d_in = x.shape
    n_exp, d_out, _ = W.shape
    assert d_in == D_IN and d_out == D_OUT and n_exp == N_EXPERTS

    xf = np.ascontiguousarray(x, dtype=np.float32).reshape(-1, d_in)
    p = partitions.reshape(-1)

    tok_ids = [np.nonzero(p == e)[0] for e in range(N_EXPERTS)]
    max_cnt = max(len(ids) for ids in tok_ids)
    C = max(NBLK, ((max_cnt + P - 1) // P) * P)

    in_maps = []
    for e in range(N_EXPERTS):
        ids = tok_ids[e]
        xT = np.zeros((D_IN, C), np_dt)
        xT[:, : len(ids)] = xf[ids].T.astype(np_dt)
        in_maps.append(
            {
                "xT": xT,
                "wT": np.ascontiguousarray(W[e].T).astype(np_dt),
                "bias": np.ascontiguousarray(b[e], dtype=np.float32),
            }
        )

    nc = _get_nc(C, math_mode)
    res = run_bass_kernel_spmd(nc, in_maps, list(range(N_CORES)), trace=_trace)

    outf = np.empty((B * S, d_out), np.float32)
    for e in range(N_EXPERTS):
        ids = tok_ids[e]
        outf[ids] = np.asarray(res.results[e]["out"])[:, : len(ids)].T.astype(
            np.float32
        )
    out = outf.reshape(B, S, d_out)
    if _trace:
        return out, res
    return out
